# revision 9
# baseline (speedup 1.0000x reference)
"""KAN layer (Gaussian RBF basis + contraction) as a Bass/Tile kernel on 8 TRN2 cores.

Math: out[b,o] = sum_{i,k} exp(-(x[b,i]-centers[k])^2 / (2*widths[k]^2)) * weights[o,i,k] + bias[o]

Strategy: data-parallel over batch B=4096 -> 512 rows/core; weights replicated.
Per core this is G[512, 16384] @ Wt[16384, 1024] with the basis G computed
on-chip (ScalarE Square+Exp) in contraction-major layout so it feeds the
PE array directly as the stationary operand. Contraction dim ordered k-major
(c = k*1024 + i) so each 128-row chunk of the contraction is one (k, i_block)
pair: G chunk = elementwise gaussian of a resident x^T tile, W chunk = 128
contiguous DRAM rows of the host-pretransposed weights.

The full per-core output [512, 1024] fp32 lives in all 8 PSUM banks and is
accumulated across 128 chunk matmul-groups, then bias-added on DVE on the way
out.
"""

import os
from functools import lru_cache

import numpy as np
import ml_dtypes

import concourse.bass as bass
import concourse.mybir as mybir
import concourse.tile as tile
from concourse import bacc
from concourse.bass_utils import run_bass_kernel_spmd

B, I, O, K = 4096, 1024, 1024, 16
N_CORES = 8
BS = B // N_CORES          # 512 batch rows per core
IB = I // 128              # 8 i-blocks
NCHUNK = K * IB            # 128 contraction chunks of 128
F32 = mybir.dt.float32

# matmul operand dtype: "bf16" | "f32r" | "f32"
MM_DTYPE = os.environ.get("KERNEL_MM_DTYPE", "bf16")


def _mm_dt():
    return {
        "bf16": mybir.dt.bfloat16,
        "f32r": mybir.dt.float32r,
        "f32": mybir.dt.float32,
    }[MM_DTYPE]


def _np_store_dt():
    # numpy dtype used for the host-side weight array fed to the device
    return ml_dtypes.bfloat16 if MM_DTYPE == "bf16" else np.float32


@lru_cache(maxsize=4)
def _build_nc(centers_key: bytes, widths_key: bytes, mm_dtype_name: str):
    centers = np.frombuffer(centers_key, dtype=np.float32)
    widths = np.frombuffer(widths_key, dtype=np.float32)
    mdt = _mm_dt()

    nc = bacc.Bacc("TRN2", target_bir_lowering=False, debug=False)

    xt = nc.dram_tensor("xt", [I, BS], F32, kind="ExternalInput")
    wt = nc.dram_tensor("wt", [K * I, O], mdt, kind="ExternalInput")
    bias_b = nc.dram_tensor("biasb", [128, O], F32, kind="ExternalInput")
    cents = nc.dram_tensor("cents", [128, K], F32, kind="ExternalInput")
    out = nc.dram_tensor("out", [BS, O], F32, kind="ExternalOutput")

    with tile.TileContext(nc) as tc:
        with (
            tc.tile_pool(name="const", bufs=1) as const_pool,
            tc.tile_pool(name="w", bufs=3) as w_pool,
            tc.tile_pool(name="g", bufs=3) as g_pool,
            tc.tile_pool(name="d2", bufs=2) as d2_pool,
            tc.tile_pool(name="o", bufs=2) as o_pool,
            tc.tile_pool(name="psum", bufs=1, space=bass.MemorySpace.PSUM) as psum_pool,
        ):
            # resident x^T: [128 part, 8 i-blocks, 512 batch] fp32 (2MB)
            xt_t = const_pool.tile([128, IB, BS], F32)
            nc.sync.dma_start(xt_t[:], xt.rearrange("(ib p) b -> p ib b", p=128))
            bias_t = const_pool.tile([128, O], F32)
            nc.sync.dma_start(bias_t[:], bias_b[:])
            cents_t = const_pool.tile([128, K], F32)
            nc.sync.dma_start(cents_t[:], cents[:])

            # whole per-core output accumulates in PSUM: 8 banks of [128, 512]
            psum_t = psum_pool.tile([128, 2 * IB // 2, BS], F32)  # [128, 8, 512]

            for c in range(NCHUNK):
                k, ib = divmod(c, IB)
                w_t = w_pool.tile([128, O], mdt)
                nc.sync.dma_start(w_t[:], wt[c * 128:(c + 1) * 128, :])

                d2 = d2_pool.tile([128, BS], F32)
                nc.scalar.activation(
                    d2[:], xt_t[:, ib, :],
                    mybir.ActivationFunctionType.Square,
                    bias=cents_t[:, k:k + 1], scale=1.0,
                )
                g = g_pool.tile([128, BS], mdt)
                nc.scalar.activation(
                    g[:], d2[:],
                    mybir.ActivationFunctionType.Exp,
                    bias=0.0, scale=-1.0 / (2.0 * float(widths[k]) ** 2),
                )

                first, last = c == 0, c == NCHUNK - 1
                for m in range(BS // 128):          # 4 output row blocks
                    for n in range(O // 512):       # 2 output col halves
                        nc.tensor.matmul(
                            psum_t[:, m * 2 + n, :],
                            g[:, m * 128:(m + 1) * 128],
                            w_t[:, n * 512:(n + 1) * 512],
                            start=first, stop=last,
                        )

            # PSUM -> SBUF with bias add, then DMA out
            for m in range(BS // 128):
                o_t = o_pool.tile([128, O], F32)
                for n in range(O // 512):
                    nc.vector.tensor_add(
                        o_t[:, n * 512:(n + 1) * 512],
                        psum_t[:, m * 2 + n, :],
                        bias_t[:, n * 512:(n + 1) * 512],
                    )
                nc.sync.dma_start(out[m * 128:(m + 1) * 128, :], o_t[:])

    nc.compile()
    return nc


def kernel(x, weights, bias, centers, widths):
    x = np.asarray(x, dtype=np.float32)
    weights = np.asarray(weights, dtype=np.float32)
    bias = np.asarray(bias, dtype=np.float32)
    centers = np.asarray(centers, dtype=np.float32)
    widths = np.asarray(widths, dtype=np.float32)

    nc = _build_nc(centers.tobytes(), widths.tobytes(), MM_DTYPE)

    # host-side prep: x^T shards, k-major transposed weights (replicated)
    xt = np.ascontiguousarray(x.T)                                  # [I, B]
    wt = np.ascontiguousarray(
        weights.transpose(2, 1, 0).reshape(K * I, O)                # [(k,i), o]
    ).astype(_np_store_dt())
    bias_b = np.ascontiguousarray(np.broadcast_to(bias, (128, O)))
    cents_b = np.ascontiguousarray(np.broadcast_to(-centers, (128, K)))

    in_maps = [
        {
            "xt": np.ascontiguousarray(xt[:, c * BS:(c + 1) * BS]),
            "wt": wt,
            "biasb": bias_b,
            "cents": cents_b,
        }
        for c in range(N_CORES)
    ]

    res = run_bass_kernel_spmd(
        nc, in_maps, list(range(N_CORES)),
        trace=bool(os.environ.get("KERNEL_TRACE")),
        tmpdir=os.environ.get("KERNEL_TMPDIR"),
    )
    kernel.last_results = res

    return np.concatenate(
        [res.results[c]["out"] for c in range(N_CORES)], axis=0
    ).astype(np.float32)
